# revision 15
# baseline (speedup 1.0000x reference)
"""RNN-T joint network kernel for 8 trn2 NeuronCores.

out[b,t,u,v] = sum_d (x[b,t,d] * y[b,u,d]) * W[v,d] + b[v]
B=4, T=256, U=64, D=256, V=1024, fp32.

Sharding: T split 8 ways (32 t's per core). Each core computes its
(4, 32, 64, 1024) output slice (32 MB); host concatenates on axis 1.

Per-core plan (output-DMA bound, ~32 MB written per core):
  - host pre-transposes x->(B,D,Ts), y->(B,D,U), W->(D,V) so all SBUF
    loads are contiguous with d on the partition dim; bias replicated
    to (128, V) on host.
  - per (b, t-pair): DVE builds the stationary Z^T[d,(t,u)] tile
    (y^T column-block times per-partition scalar x[b,t,d], rounded to
    fp32r), PE does 4 fp32r matmuls (K=128 x2 accum, M=128=(2t x 64u),
    N=512 x2) into one 2-bank PSUM tile, DVE evacuates PSUM->SBUF
    fused with the bias add, one 512 KB contiguous DMA per t-pair.

Toolchain quirk: this walrus build rejects >1 embedded semaphore wait
per engine instruction ("Too many sync wait commands") — even the
in-repo production matmul_tile_kernel trips it. split_multi_waits()
post-processes the scheduled BIR, hoisting all but one wait of each
instruction onto same-engine NoOps placed directly before it (the
standalone-wait encoding), which is semantically identical.
"""

import sys

sys.path.insert(0, "/opt/trn_rl_repo")

from contextlib import ExitStack

import numpy as np

import concourse.bass as bass
import concourse.mybir as mybir
import concourse.tile as tile
from concourse.bass_utils import run_bass_kernel_spmd

B, T, U, D, V = 4, 256, 64, 256, 1024
NCORES = 8
TS = T // NCORES  # 32
FP32 = mybir.dt.float32
FP32R = mybir.dt.float32r

_CACHED_NC = None


def split_multi_waits(nc):
    """Hoist all but one embedded wait per instruction onto NoOps."""
    n_split = 0
    for fn in nc.m.functions:
        for bb in fn.blocks:
            new_insts = []
            for inst in bb.instructions:
                si = inst.sync_info
                if si is not None and si.on_wait is not None and len(si.on_wait) > 1:
                    waits = list(si.on_wait)
                    for w in waits[:-1]:
                        nop = mybir.InstNoOp(
                            name=nc.get_next_instruction_name(),
                            text_hint="waitsplit",
                        )
                        nop.engine = inst.engine
                        nop.sync_info = mybir.SyncInfo(on_wait=[w], on_update=[])
                        nc.register_instruction(nop)
                        new_insts.append(nop)
                        n_split += 1
                    inst.sync_info = mybir.SyncInfo(
                        on_wait=[waits[-1]], on_update=list(si.on_update or [])
                    )
                new_insts.append(inst)
            bb.instructions = new_insts
    return n_split


def build_nc():
    nc = bass.Bass()
    xT = nc.declare_dram_parameter("xT", [B, D, TS], FP32, isOutput=False)
    yT = nc.declare_dram_parameter("yT", [B, D, U], FP32, isOutput=False)
    wT = nc.declare_dram_parameter("wT", [D, V], FP32, isOutput=False)
    bR = nc.declare_dram_parameter("bR", [128, V], FP32, isOutput=False)
    out = nc.declare_dram_parameter("out", [B, TS, U, V], FP32, isOutput=True)

    with tile.TileContext(nc) as tc, ExitStack() as ctx:
        consts = ctx.enter_context(tc.tile_pool(name="consts", bufs=1))
        zpool = ctx.enter_context(tc.tile_pool(name="z", bufs=3))
        pspool = ctx.enter_context(tc.tile_pool(name="ps", bufs=3, space="PSUM"))
        opool = ctx.enter_context(tc.tile_pool(name="o", bufs=4))

        # Constant tiles, loaded once; d (contraction) on partitions.
        # SWDGE (gpsimd) DMAs: single completion semaphore each, and the
        # w load needs the SWDGE fp32 -> fp32r cast (fp32r consumers
        # require a converting producer, not a bitcast).
        w_sb = consts.tile([128, 2 * V], FP32R, tag="w")  # [:, dc*V + v]
        nc.gpsimd.dma_start(
            w_sb[:].rearrange("p (dc v) -> p dc v", dc=2),
            wT.rearrange("(dc p) v -> p dc v", dc=2),
        )
        b_sb = consts.tile([128, V], FP32, tag="b")
        nc.gpsimd.dma_start(b_sb[:], bR[:, :])
        y_sb = consts.tile([128, B * 2 * U], FP32, tag="y")  # [:, (b*2+dc)*U + u]
        nc.gpsimd.dma_start(
            y_sb[:].rearrange("p (b dc u) -> p b dc u", b=B, dc=2),
            yT.rearrange("b (dc p) u -> p b dc u", dc=2),
        )
        x_sb = consts.tile([128, B * 2 * TS], FP32, tag="x")  # [:, (b*2+dc)*TS + t]
        nc.gpsimd.dma_start(
            x_sb[:].rearrange("p (b dc t) -> p b dc t", b=B, dc=2),
            xT.rearrange("b (dc p) t -> p b dc t", dc=2),
        )

        for b in range(B):
            for tp in range(TS // 2):
                # Stationary Z^T: columns m = ti*64+u, partition = d (chunk dc).
                zz = zpool.tile([128, 256], FP32R, tag="zz")
                for dc in range(2):
                    for ti in range(2):
                        t = 2 * tp + ti
                        nc.vector.tensor_scalar_mul(
                            zz[:, dc * 128 + ti * U : dc * 128 + (ti + 1) * U],
                            y_sb[:, (b * 2 + dc) * U : (b * 2 + dc + 1) * U],
                            x_sb[:, (b * 2 + dc) * TS + t : (b * 2 + dc) * TS + t + 1],
                        )
                ps = pspool.tile([128, V], FP32, tag="ps")
                for n in range(2):
                    for dc in range(2):
                        nc.tensor.matmul(
                            ps[:, n * 512 : (n + 1) * 512],
                            zz[:, dc * 128 : (dc + 1) * 128],
                            w_sb[:, dc * V + n * 512 : dc * V + (n + 1) * 512],
                            start=(dc == 0),
                            stop=(dc == 1),
                        )
                ot = opool.tile([128, V], FP32, tag="ot")
                nc.vector.tensor_add(ot[:], ps[:], b_sb[:])
                nc.sync.dma_start(
                    out[b, 2 * tp : 2 * tp + 2].rearrange("t u v -> (t u) v"),
                    ot[:],
                )

    split_multi_waits(nc)
    return nc


def _get_nc():
    global _CACHED_NC
    if _CACHED_NC is None:
        _CACHED_NC = build_nc()
    return _CACHED_NC


def _make_in_maps(x, y, W, b):
    x = np.asarray(x, dtype=np.float32)
    y = np.asarray(y, dtype=np.float32)
    W = np.asarray(W, dtype=np.float32)
    b = np.asarray(b, dtype=np.float32)
    xTfull = np.ascontiguousarray(x.transpose(0, 2, 1))  # (B, D, T)
    yT = np.ascontiguousarray(y.transpose(0, 2, 1))  # (B, D, U)
    wT = np.ascontiguousarray(W.T)  # (D, V)
    bR = np.ascontiguousarray(np.broadcast_to(b[None, :], (128, V)))
    return [
        {
            "xT": np.ascontiguousarray(xTfull[:, :, i * TS : (i + 1) * TS]),
            "yT": yT,
            "wT": wT,
            "bR": bR,
        }
        for i in range(NCORES)
    ]


def run(x, y, W, b, trace=False, **spmd_kwargs):
    res = run_bass_kernel_spmd(
        _get_nc(),
        _make_in_maps(x, y, W, b),
        list(range(NCORES)),
        trace=trace,
        **spmd_kwargs,
    )
    out = np.concatenate([res.results[i]["out"] for i in range(NCORES)], axis=1)
    return out, res


def kernel(x, y, W, b):
    out, _ = run(x, y, W, b)
    return out


# revision 30
# speedup vs baseline: 729.2196x; 729.2196x over previous
"""RNN-T joint network kernel for 8 trn2 NeuronCores.

out[b,t,u,v] = sum_d (x[b,t,d] * y[b,u,d]) * W[v,d] + b[v]
B=4, T=256, U=64, D=256, V=1024, fp32.

Sharding: T split 8 ways (32 t's per core). Each core computes its
(4, 32, 64, 1024) output slice (32 MB); host concatenates on axis 1.

Per-core plan (output-DMA bound, ~32 MB written per core):
  - host pre-transposes x->(B,D,Ts), y->(B,D,U), W->(D,V) so all SBUF
    loads are contiguous with d on the partition dim; bias replicated
    to (128, V) on host.
  - per (b, t-pair): DVE builds the stationary Z^T[d,(t,u)] tile
    (y^T column-block times per-partition scalar x[b,t,d], rounded to
    fp32r), PE does 4 fp32r matmuls (K=128 x2 accum, M=128=(2t x 64u),
    N=512 x2) into one 2-bank PSUM tile, DVE evacuates PSUM->SBUF
    fused with the bias add, one 512 KB contiguous DMA per t-pair.

Toolchain quirk: this walrus build rejects >1 embedded semaphore wait
per engine instruction ("Too many sync wait commands") — even the
in-repo production matmul_tile_kernel trips it. split_multi_waits()
post-processes the scheduled BIR, hoisting all but one wait of each
instruction onto same-engine NoOps placed directly before it (the
standalone-wait encoding), which is semantically identical.
"""

import sys

sys.path.insert(0, "/opt/trn_rl_repo")

from contextlib import ExitStack

import numpy as np

import concourse.bass as bass
import concourse.mybir as mybir
import concourse.tile as tile
from concourse.bass_utils import run_bass_kernel_spmd

B, T, U, D, V = 4, 256, 64, 256, 1024
NCORES = 8
TS = T // NCORES  # 32
FP32 = mybir.dt.float32
FP32R = mybir.dt.float32r

_CACHED_NC = None


def split_multi_waits(nc):
    """Hoist all but one embedded wait per instruction onto NoOps."""
    n_split = 0
    for fn in nc.m.functions:
        for bb in fn.blocks:
            new_insts = []
            for inst in bb.instructions:
                si = inst.sync_info
                if si is not None and si.on_wait is not None and len(si.on_wait) > 1:
                    waits = list(si.on_wait)
                    for w in waits[:-1]:
                        nop = mybir.InstNoOp(
                            name=nc.get_next_instruction_name(),
                            text_hint="waitsplit",
                        )
                        nop.engine = inst.engine
                        nop.sync_info = mybir.SyncInfo(on_wait=[w], on_update=[])
                        nc.register_instruction(nop)
                        new_insts.append(nop)
                        n_split += 1
                    inst.sync_info = mybir.SyncInfo(
                        on_wait=[waits[-1]], on_update=list(si.on_update or [])
                    )
                new_insts.append(inst)
            bb.instructions = new_insts
    return n_split


def build_nc():
    nc = bass.Bass()
    xT = nc.declare_dram_parameter("xT", [B, D, TS], FP32, isOutput=False)
    yT = nc.declare_dram_parameter("yT", [B, D, U], FP32, isOutput=False)
    wT = nc.declare_dram_parameter("wT", [D, V], FP32R, isOutput=False)
    bR = nc.declare_dram_parameter("bR", [1, V], FP32R, isOutput=False)
    bRep = nc.declare_dram_parameter("bRep", [128, V], FP32, isOutput=False)
    onesD = nc.declare_dram_parameter("onesD", [1, 128], FP32R, isOutput=False)
    out = nc.declare_dram_parameter("out", [B, TS, U, V], FP32, isOutput=True)

    with tile.TileContext(nc) as tc, ExitStack() as ctx:
        consts = ctx.enter_context(tc.tile_pool(name="consts", bufs=1))
        zpool = ctx.enter_context(tc.tile_pool(name="z", bufs=4))
        pspool = ctx.enter_context(tc.tile_pool(name="ps", bufs=3, space="PSUM"))
        wppool = ctx.enter_context(tc.tile_pool(name="wp", bufs=1, space="PSUM"))
        opool = ctx.enter_context(tc.tile_pool(name="o", bufs=6))

        # Constant tiles, loaded once; d (contraction) on partitions.
        # SWDGE (gpsimd) DMAs: single completion semaphore each, and the
        # w load needs the SWDGE fp32 -> fp32r cast (fp32r consumers
        # require a converting producer, not a bitcast).
        # Bulk loads ride HWDGE (fast start); the fp32r conversions
        # happen on-chip (ScalarE) from fp32 staging tiles, since only
        # SWDGE can cast in-flight and SWDGE serializes ~1us/DMA on Q7.
        # Load order = first-use order, so the pipeline fills ASAP.
        # Inputs arrive already in fp32r bit layout (fp32r at rest is
        # plain fp32 bits — HW-verified), so everything is a plain
        # HWDGE load: no staging, no casts. Tiny rows first (enable PE
        # warmup + bias matmuls), then y/x (z-prep), W, replicated bias.
        ones_r = consts.tile([1, 128], FP32R, tag="ones")
        nc.sync.dma_start(ones_r[:], onesD[:, :])
        b_r = consts.tile([1, V], FP32R, tag="br")
        nc.sync.dma_start(b_r[:], bR[0:1, :])
        y_sb = consts.tile([128, B * 2 * U], FP32, tag="y")  # [:, (b*2+dc)*U + u]
        nc.sync.dma_start(
            y_sb[:].rearrange("p (b dc u) -> p b dc u", b=B, dc=2),
            yT.rearrange("b (dc p) u -> p b dc u", dc=2),
        )
        x_sb = consts.tile([128, B * 2 * TS], FP32, tag="x")  # [:, (b*2+dc)*TS + t]
        nc.sync.dma_start(
            x_sb[:].rearrange("p (b dc t) -> p b dc t", b=B, dc=2),
            xT.rearrange("b (dc p) t -> p b dc t", dc=2),
        )
        w_sb = consts.tile([128, 2 * V], FP32R, tag="w")  # [:, dc*V + v]
        for dc in range(2):
            nc.sync.dma_start(
                w_sb[:, dc * V : (dc + 1) * V], wT[dc * 128 : (dc + 1) * 128, :]
            )
        b_sb = consts.tile([128, V], FP32, tag="b")
        nc.sync.dma_start(b_sb[:], bRep[:, :])
        # PE warmup: K=1 dummy matmuls keep the PE active through the
        # load phase so the first real tiles run at full clock.
        wps = wppool.tile([128, 512], FP32, tag="wps")
        for _ in range(6):
            nc.tensor.matmul(wps[:, :], ones_r[:, :], b_r[:, 0:512], start=True, stop=True)

        for b in range(B):
            for tp in range(TS // 2):
                it = b * (TS // 2) + tp
                use_act = it % 2 == 0
                # Stationary Z^T: columns m = ti*64+u, partition = d (chunk dc).
                zz = zpool.tile([128, 256], FP32R, tag="zz")
                for dc in range(2):
                    for ti in range(2):
                        t = 2 * tp + ti
                        nc.vector.tensor_scalar_mul(
                            zz[:, dc * 128 + ti * U : dc * 128 + (ti + 1) * U],
                            y_sb[:, (b * 2 + dc) * U : (b * 2 + dc + 1) * U],
                            x_sb[:, (b * 2 + dc) * TS + t : (b * 2 + dc) * TS + t + 1],
                        )
                ps = pspool.tile([128, V], FP32, tag="ps")
                for n in range(2):
                    for dc in range(2):
                        nc.tensor.matmul(
                            ps[:, n * 512 : (n + 1) * 512],
                            zz[:, dc * 128 : (dc + 1) * 128],
                            w_sb[:, dc * V + n * 512 : dc * V + (n + 1) * 512],
                            start=(dc == 0),
                            stop=(dc == 1) and not use_act,
                        )
                    if use_act:
                        # += ones^T @ bias_row: bias lands in PSUM so
                        # ScalarE can evacuate with a plain copy.
                        nc.tensor.matmul(
                            ps[:, n * 512 : (n + 1) * 512],
                            ones_r[:, :],
                            b_r[:, n * 512 : (n + 1) * 512],
                            start=False,
                            stop=True,
                        )
                ot = opool.tile([128, V], FP32, tag="ot")
                if use_act:
                    nc.scalar.copy(ot[:], ps[:])
                else:
                    nc.vector.tensor_add(ot[:], ps[:], b_sb[:])
                nc.sync.dma_start(
                    out[b, 2 * tp : 2 * tp + 2].rearrange("t u v -> (t u) v"),
                    ot[:],
                )

    split_multi_waits(nc)
    return nc


def _get_nc():
    global _CACHED_NC
    if _CACHED_NC is None:
        _CACHED_NC = build_nc()
    return _CACHED_NC


def _make_in_maps(x, y, W, b):
    x = np.asarray(x, dtype=np.float32)
    y = np.asarray(y, dtype=np.float32)
    W = np.asarray(W, dtype=np.float32)
    b = np.asarray(b, dtype=np.float32)
    xTfull = np.ascontiguousarray(x.transpose(0, 2, 1))  # (B, D, T)
    yT = np.ascontiguousarray(y.transpose(0, 2, 1))  # (B, D, U)
    wT = np.ascontiguousarray(W.T)  # (D, V)
    bRow = np.ascontiguousarray(b[None, :])  # (1, V)
    bRep = np.ascontiguousarray(np.broadcast_to(b[None, :], (128, V)))
    return [
        {
            "xT": np.ascontiguousarray(xTfull[:, :, i * TS : (i + 1) * TS]),
            "yT": yT,
            "wT": wT,
            "bR": bRow,
            "bRep": bRep,
            "onesD": np.ones((1, 128), np.float32),
        }
        for i in range(NCORES)
    ]


def run(x, y, W, b, trace=False, **spmd_kwargs):
    res = run_bass_kernel_spmd(
        _get_nc(),
        _make_in_maps(x, y, W, b),
        list(range(NCORES)),
        trace=trace,
        **spmd_kwargs,
    )
    out = np.concatenate([res.results[i]["out"] for i in range(NCORES)], axis=1)
    return out, res


def kernel(x, y, W, b):
    out, _ = run(x, y, W, b)
    return out
